# revision 20
# baseline (speedup 1.0000x reference)
"""DIEN GRU (dynamic_rnn + GRUCell + sequence_length masking) on TRN2.

Strategy:
 - B=1024 rows are sorted by seq_len (desc) and dealt round-robin to TWO
   cores (512 rows each, near-identical length profiles). The per-step
   instruction count is width-independent, so fewer/wider cores minimize the
   serial instruction stream; two cores (not one) keep the alive prefix
   k <= 512 so every gate is a single matmul per step (one PSUM bank).
 - At step t only the alive prefix of k_t columns is computed; the output
   DMA per step covers exactly the shared alive prefix. The <=1-column
   alive-count mismatch between the two cores plus anything beyond a row's
   seq_len is zeroed on the HOST after gathering (y[b, t>=L_b] = 0), so the
   device program needs no masks, memsets, or state-holding.
 - Channels on partitions (h-dim = partition), batch on the free dim.
 - Per step (k = shared alive prefix, <= 512):
     pre_r = Wx_r@x + Wh_r@h          (PSUM accumulation)
     pre_u = Wx_u@x + Wh_u@h
     r = sigmoid(pre_r + br)          (bias folded into ACT as [H,1] AP)
     v = sigmoid(-pre_u - bu) = 1-u   (scale=-1 trick)
     pre_c = Wc_x@x + Wc_h@(r*h); c = tanh(pre_c + bc)   (r*h in-place)
     q = v*c ; p = (v-1)*h ; h' = q - p  (== u*h + (1-u)*c)
 - State h is kept in fp16 (tolerance is 2e-2; measured error ~1e-3).
 - x is packed host-side per chunk with stride k0 (the first step's rounded
   prefix), halving both upload and on-device DMA bytes.
"""


import numpy as np

B, T, D, H = 1024, 200, 128, 128
N_CORES = 8      # platform cores
N_ACTIVE = 2     # cores doing compute
BL = B // N_ACTIVE
CH = 16          # time steps per x-DMA chunk
KR = 8           # round alive-prefix up to a multiple of this

_compiled_cache: dict = {}
_runner_cache: dict = {}
_prep_cache: dict = {}


def _round_up(x, m):
    return ((x + m - 1) // m) * m


def _schedule(seq_len):
    order = np.argsort(-seq_len, kind="stable")
    perms = [order[c::N_ACTIVE] for c in range(N_ACTIVE)]
    t_eff = int(seq_len.max()) if seq_len.size else 0
    # shared alive schedule: max over cores of the per-core alive count
    k_true = np.zeros(T, dtype=np.int64)
    for p in perms:
        Lc = seq_len[p]
        kc = (Lc[:, None] > np.arange(T)[None, :]).sum(axis=0)
        k_true = np.maximum(k_true, kc)
    k_round = np.minimum(_round_up(k_true, KR), BL)
    chunks = []
    t0 = 0
    while t0 < t_eff:
        ns = min(CH, t_eff - t0)
        chunks.append((t0, ns, int(k_round[t0])))
        t0 += ns
    return perms, tuple(int(v) for v in k_true), tuple(int(v) for v in k_round), \
        t_eff, tuple(chunks)


def _prepare(inputs):
    x = np.asarray(inputs["item_his_eb"], dtype=np.float32)
    seq_len = np.asarray(inputs["seq_len"], dtype=np.int32)
    W_gate = np.asarray(inputs["W_gate"], dtype=np.float32)
    b_gate = np.asarray(inputs["b_gate"], dtype=np.float32)
    W_cand = np.asarray(inputs["W_cand"], dtype=np.float32)
    b_cand = np.asarray(inputs["b_cand"], dtype=np.float32)

    perms, k_true, k_round, t_eff, chunks = _schedule(seq_len)

    common = {
        "wgx": W_gate[0:D, :].astype(np.float16),
        "wgh": W_gate[D:D + H, :].astype(np.float16),
        "wcx": W_cand[0:D, :].astype(np.float16),
        "wch": W_cand[D:D + H, :].astype(np.float16),
        "br": b_gate[0:H].reshape(H, 1).astype(np.float32),
        "bun": (-b_gate[H:2 * H]).reshape(H, 1).astype(np.float32),
        "bc": b_cand.reshape(H, 1).astype(np.float32),
    }
    in_maps = []
    for p in perms:
        xa = x[p].transpose(2, 1, 0)  # [D, T, BL] (view)
        slabs = [np.ascontiguousarray(xa[:, t0:t0 + ns, :k0]).reshape(D, ns * k0)
                 for (t0, ns, k0) in chunks]
        if slabs:
            xq = np.concatenate(slabs, axis=1).astype(np.float16)
        else:
            xq = np.zeros((D, KR), dtype=np.float16)
        in_maps.append({"xq": xq, **common})

    xlen = int(in_maps[0]["xq"].shape[1])
    sched = (k_true, k_round, t_eff, chunks, xlen)
    return in_maps, perms, seq_len, sched


def _build_program(sched, repeat=1, opts=()):
    opts = dict(opts)
    nbuf = opts.get("nbuf", 2)
    nbuf_h = opts.get("nbuf_h", 2)
    gp_sub = opts.get("gp_sub", False)
    inplace_rh = opts.get("inplace_rh", True)
    ydma_eng = opts.get("ydma_eng", "scalar")
    from contextlib import ExitStack

    import concourse.tile as tile
    from concourse import bacc, mybir

    k_true, k_round, t_eff, chunks, xlen = sched
    f32 = mybir.dt.float32
    f16 = mybir.dt.float16
    Sig = mybir.ActivationFunctionType.Sigmoid
    Tanh = mybir.ActivationFunctionType.Tanh
    Alu = mybir.AluOpType

    nc = bacc.Bacc("TRN2", target_bir_lowering=False, debug=False,
                   num_devices=N_ACTIVE)

    xq_d = nc.dram_tensor("xq", [D, xlen], f16, kind="ExternalInput").ap()
    wgx_d = nc.dram_tensor("wgx", [D, 2 * H], f16, kind="ExternalInput").ap()
    wgh_d = nc.dram_tensor("wgh", [H, 2 * H], f16, kind="ExternalInput").ap()
    wcx_d = nc.dram_tensor("wcx", [D, H], f16, kind="ExternalInput").ap()
    wch_d = nc.dram_tensor("wch", [H, H], f16, kind="ExternalInput").ap()
    br_d = nc.dram_tensor("br", [H, 1], f32, kind="ExternalInput").ap()
    bun_d = nc.dram_tensor("bun", [H, 1], f32, kind="ExternalInput").ap()
    bc_d = nc.dram_tensor("bc", [H, 1], f32, kind="ExternalInput").ap()
    yT_d = nc.dram_tensor("yT", [H, T * BL], f16, kind="ExternalOutput").ap()

    any_hi = any(k > 512 for k in k_round[:t_eff])

    with tile.TileContext(nc) as tc:
        with ExitStack() as ctx:
            wpool = ctx.enter_context(tc.tile_pool(name="w", bufs=1))
            xpool = ctx.enter_context(tc.tile_pool(name="x", bufs=2))
            hpool = ctx.enter_context(tc.tile_pool(name="h", bufs=nbuf_h))
            rpool = ctx.enter_context(tc.tile_pool(name="r", bufs=nbuf))
            vpool = ctx.enter_context(tc.tile_pool(name="v", bufs=nbuf))
            cpool = ctx.enter_context(tc.tile_pool(name="c", bufs=nbuf))
            rhpool = ctx.enter_context(tc.tile_pool(name="rh", bufs=nbuf))
            qpool = ctx.enter_context(tc.tile_pool(name="q", bufs=nbuf))
            ppool = ctx.enter_context(tc.tile_pool(name="p", bufs=nbuf))
            pw = 1024 if any_hi else 512
            prp = ctx.enter_context(tc.tile_pool(name="prp", bufs=1, space="PSUM"))
            pvp = ctx.enter_context(tc.tile_pool(name="pvp", bufs=1, space="PSUM"))
            pcp = ctx.enter_context(tc.tile_pool(name="pcp", bufs=1, space="PSUM"))

            wgx = wpool.tile([D, 2 * H], f16)
            nc.sync.dma_start(wgx[:], wgx_d[:])
            wgh = wpool.tile([H, 2 * H], f16)
            nc.sync.dma_start(wgh[:], wgh_d[:])
            wcx = wpool.tile([D, H], f16)
            nc.sync.dma_start(wcx[:], wcx_d[:])
            wch = wpool.tile([H, H], f16)
            nc.sync.dma_start(wch[:], wch_d[:])
            br = wpool.tile([H, 1], f32)
            nc.sync.dma_start(br[:], br_d[:])
            bun = wpool.tile([H, 1], f32)
            nc.sync.dma_start(bun[:], bun_d[:])
            bc = wpool.tile([H, 1], f32)
            nc.sync.dma_start(bc[:], bc_d[:])

            # initialize every h-chunk buffer once so the whole-chunk y-DMA
            # never reads uninitialized SBUF (stale finite values are fine:
            # the host zeroes every t >= seq_len cell after gathering)
            hmax = max((ns for (_t0, ns, _k0) in chunks), default=1) * BL
            for _hb in range(nbuf_h):
                hz = hpool.tile([128, hmax], f16)
                nc.gpsimd.memset(hz[:], 0.0)

            for _rep in range(repeat):
                h_prev = None
                xoff = 0
                for (t0, ns, k0) in chunks:
                    xc = xpool.tile([128, ns * k0], f16)
                    nc.scalar.dma_start(xc[:], xq_d[:, xoff: xoff + ns * k0])
                    hc = hpool.tile([128, hmax], f16)
                    rc = rpool.tile([128, ns * BL], f16)
                    vc = vpool.tile([128, ns * BL], f16)
                    cc = cpool.tile([128, ns * BL], f16)
                    for j in range(ns):
                        t = t0 + j
                        k = k_round[t]
                        lo = min(k, 512)
                        hi = k - lo
                        xs = xc[:, j * k0: j * k0 + k]

                        rp = prp.tile([128, pw], f32)
                        vp = pvp.tile([128, pw], f32)
                        cp = pcp.tile([128, pw], f32)

                        first = (t == 0)
                        # x contributions (weight-adjacent order)
                        nc.tensor.matmul(rp[:, 0:lo], wgx[:, 0:H], xs[:, 0:lo],
                                         start=True, stop=first)
                        if hi:
                            nc.tensor.matmul(rp[:, 512:k], wgx[:, 0:H],
                                             xs[:, 512:k], start=True, stop=first)
                        nc.tensor.matmul(vp[:, 0:lo], wgx[:, H:2 * H], xs[:, 0:lo],
                                         start=True, stop=first)
                        if hi:
                            nc.tensor.matmul(vp[:, 512:k], wgx[:, H:2 * H],
                                             xs[:, 512:k], start=True, stop=first)
                        nc.tensor.matmul(cp[:, 0:lo], wcx[:], xs[:, 0:lo],
                                         start=True, stop=first)
                        if hi:
                            nc.tensor.matmul(cp[:, 512:k], wcx[:],
                                             xs[:, 512:k], start=True, stop=first)
                        # recurrent gate contributions
                        if not first:
                            nc.tensor.matmul(rp[:, 0:lo], wgh[:, 0:H],
                                             h_prev[:, 0:lo], start=False, stop=True)
                            if hi:
                                nc.tensor.matmul(rp[:, 512:k], wgh[:, 0:H],
                                                 h_prev[:, 512:k], start=False,
                                                 stop=True)
                            nc.tensor.matmul(vp[:, 0:lo], wgh[:, H:2 * H],
                                             h_prev[:, 0:lo], start=False, stop=True)
                            if hi:
                                nc.tensor.matmul(vp[:, 512:k], wgh[:, H:2 * H],
                                                 h_prev[:, 512:k], start=False,
                                                 stop=True)

                        r16 = rc[:, j * BL: (j + 1) * BL]
                        nc.scalar.activation(r16[:, 0:k], rp[:, 0:k], Sig,
                                             bias=br[:])
                        v16 = vc[:, j * BL: (j + 1) * BL]
                        nc.scalar.activation(v16[:, 0:k], vp[:, 0:k], Sig,
                                             bias=bun[:], scale=-1.0)

                        if not first:
                            if inplace_rh:
                                rh = r16
                            else:
                                rh = rhpool.tile([128, BL], f16)
                            nc.vector.tensor_mul(rh[:, 0:k], r16[:, 0:k],
                                                 h_prev[:, 0:k])
                            nc.tensor.matmul(cp[:, 0:lo], wch[:], rh[:, 0:lo],
                                             start=False, stop=True)
                            if hi:
                                nc.tensor.matmul(cp[:, 512:k], wch[:],
                                                 rh[:, 512:k], start=False,
                                                 stop=True)

                        c16 = cc[:, j * BL: (j + 1) * BL]
                        nc.scalar.activation(c16[:, 0:k], cp[:, 0:k], Tanh,
                                             bias=bc[:])

                        h_new = hc[:, j * BL: (j + 1) * BL]
                        if first:
                            nc.vector.tensor_mul(h_new[:, 0:k], v16[:, 0:k],
                                                 c16[:, 0:k])
                        else:
                            q16 = qpool.tile([128, BL], f16)
                            nc.vector.tensor_mul(q16[:, 0:k], v16[:, 0:k],
                                                 c16[:, 0:k])
                            p16 = ppool.tile([128, BL], f16)
                            nc.vector.scalar_tensor_tensor(
                                p16[:, 0:k], v16[:, 0:k], 1.0, h_prev[:, 0:k],
                                Alu.subtract, Alu.mult)
                            if gp_sub:
                                nc.gpsimd.tensor_sub(h_new[:, 0:k], q16[:, 0:k],
                                                     p16[:, 0:k])
                            else:
                                nc.vector.tensor_sub(h_new[:, 0:k], q16[:, 0:k],
                                                     p16[:, 0:k])

                        h_prev = h_new
                    yeng = {"sync": nc.sync, "scalar": nc.scalar,
                            "gpsimd": nc.gpsimd}[ydma_eng]
                    yeng.dma_start(yT_d[:, t0 * BL: (t0 + ns) * BL],
                                   hc[:, 0:ns * BL])
                    xoff += ns * k0

    nc.compile()
    return nc


def make_runner(nc):
    """Sharded PJRT callable built once per compiled program (mesh over the
    N_ACTIVE first cores)."""
    import jax
    from jax.sharding import Mesh, PartitionSpec
    from jax.experimental.shard_map import shard_map
    from concourse import bass2jax, mybir

    bass2jax.install_neuronx_cc_hook()

    part_name = (nc.partition_id_tensor.name
                 if nc.partition_id_tensor is not None else None)
    in_names, out_names, out_avals, zero_outs = [], [], [], []
    for alloc in nc.m.functions[0].allocations:
        if not isinstance(alloc, mybir.MemoryLocationSet):
            continue
        name = alloc.memorylocations[0].name
        if alloc.kind == "ExternalInput":
            if name != part_name:
                in_names.append(name)
        elif alloc.kind == "ExternalOutput":
            shape = tuple(alloc.tensor_shape)
            dtype = mybir.dt.np(alloc.dtype)
            out_names.append(name)
            out_avals.append(jax.core.ShapedArray(shape, dtype))
            zero_outs.append(np.zeros(shape, dtype))
    n_params = len(in_names)
    all_names = in_names + out_names
    if part_name is not None:
        all_names = all_names + [part_name]

    def _body(*args):
        operands = list(args)
        if part_name is not None:
            operands.append(bass2jax.partition_id_tensor())
        outs = bass2jax._bass_exec_p.bind(
            *operands,
            out_avals=tuple(out_avals),
            in_names=tuple(all_names),
            out_names=tuple(out_names),
            lowering_input_output_aliases=(),
            sim_require_finite=False,
            sim_require_nnan=False,
            nc=nc,
        )
        return tuple(outs)

    devices = jax.devices()[:N_ACTIVE]
    mesh = Mesh(np.asarray(devices), ("core",))
    nargs = n_params + len(out_names)
    sharded = jax.jit(
        shard_map(_body, mesh=mesh,
                  in_specs=(PartitionSpec("core"),) * nargs,
                  out_specs=(PartitionSpec("core"),) * len(out_names),
                  check_rep=False),
        donate_argnums=tuple(range(n_params, nargs)), keep_unused=True)

    def run(in_maps):
        concat_in = [
            np.concatenate([np.asarray(in_maps[c][nm]) for c in
                            range(N_ACTIVE)], axis=0)
            for nm in in_names
        ]
        concat_zeros = [
            np.zeros((N_ACTIVE * z.shape[0], *z.shape[1:]), z.dtype)
            for z in zero_outs
        ]
        out_arrs = sharded(*concat_in, *concat_zeros)
        return [
            {nm: np.asarray(out_arrs[i]).reshape(
                N_ACTIVE, *out_avals[i].shape)[c]
             for i, nm in enumerate(out_names)}
            for c in range(N_ACTIVE)
        ]

    return run


def kernel(**inputs) -> np.ndarray:
    import hashlib
    hsh = hashlib.sha1()
    for name in ("item_his_eb", "seq_len", "W_gate", "b_gate", "W_cand",
                 "b_cand"):
        a = np.ascontiguousarray(np.asarray(inputs[name]))
        hsh.update(name.encode())
        hsh.update(str(a.dtype).encode())
        hsh.update(str(a.shape).encode())
        hsh.update(a.tobytes())
    pkey = hsh.hexdigest()
    prep = _prep_cache.get(pkey)
    if prep is None:
        prep = _prepare(inputs)
        _prep_cache.clear()
        _prep_cache[pkey] = prep
    in_maps, perms, seq_len, sched = prep
    k_true, k_round, t_eff, chunks, xlen = sched

    out = np.zeros((B, T, H), dtype=np.float32)
    if t_eff == 0:
        return out

    key = sched
    nc = _compiled_cache.get(key)
    if nc is None:
        nc = _build_program(sched)
        _compiled_cache.clear()
        _compiled_cache[key] = nc

    runner = _runner_cache.get(key)
    if runner is None:
        try:
            runner = make_runner(nc)
            results = runner(in_maps)
            _runner_cache.clear()
            _runner_cache[key] = runner
        except Exception:
            from concourse.bass_utils import run_bass_kernel_spmd
            runner = None
            results = run_bass_kernel_spmd(
                nc, in_maps, core_ids=list(range(N_ACTIVE))).results

    tt = np.arange(T)
    for c in range(N_ACTIVE):
        yT = results[c]["yT"]                              # [H, T*BL] f16
        yc = yT.reshape(H, T, BL).transpose(2, 1, 0).astype(np.float32)
        # zero everything at/after each row's seq_len (covers the <=1-column
        # alive-count mismatch between cores and all never-computed slots,
        # whatever garbage they hold -- np.where also clears NaN/inf)
        Lc = seq_len[perms[c]]
        valid = (tt[None, :] < Lc[:, None])[:, :, None]
        out[perms[c]] = np.where(valid, yc, 0.0)
    return out
